# revision 42
# baseline (speedup 1.0000x reference)
"""Trainium2 Bass kernel for causal self-attention with 2D RoPE.

Sharding: batch x head-group parallel over 8 NeuronCores.
  core c -> batch b = c // 4, heads h0 = (c % 4) * 3 .. h0+2.
Each core computes q/k/v projections for its 3 heads, 2D RoPE, causal
flash-attention (transposed-score layout, denominator via an appended
ones-column on V), and a per-head output projection with the softmax
normalization folded into the PSUM eviction scale. The host sums the
4 partial outputs per batch.

Matmuls run in bf16 (fp32 PSUM accumulation). All matmuls are zero-padded
to K=128 contraction so the PE activity monitor keeps the clock at 2.4GHz.
"""

import sys

sys.path.insert(0, "/opt/trn_rl_repo")

import numpy as np
from ml_dtypes import bfloat16

import concourse.bacc as bacc
import concourse.bass as bass
import concourse.mybir as mybir
from concourse import tile
from concourse.bass_utils import run_bass_kernel_spmd

BF = mybir.dt.bfloat16
F32 = mybir.dt.float32
AF = mybir.ActivationFunctionType
ALU = mybir.AluOpType

P = 128          # partitions
DM = 768         # d_model
HD = 64          # head dim
NHC = 3          # heads per core
NCC = DM // P    # contraction chunks (6)
SQT = 512        # q-block (matmul moving dim)
QKV = 3 * NHC * HD  # 576


def build_program(S=2048, n_devices=8):
    NS = S // P      # seq chunks of 128
    NQ = S // SQT    # q blocks of 512
    KPQ = SQT // P   # k-chunks per q-block (4)

    nc = bacc.Bacc(
        "TRN2", target_bir_lowering=False, debug=False, num_devices=n_devices
    )
    XB = 512
    NXB = S // XB
    xt_d = nc.dram_tensor("xt", [NXB, P, NCC, XB], BF, kind="ExternalInput")
    wqkv_d = nc.dram_tensor("wqkv", [P, NCC, QKV], BF, kind="ExternalInput")
    wo_d = nc.dram_tensor("wo", [HD, NHC, DM], BF, kind="ExternalInput")
    cos_d = nc.dram_tensor("cos", [P, S // P, HD], BF, kind="ExternalInput")
    sin_d = nc.dram_tensor("sin", [P, S // P, HD], BF, kind="ExternalInput")
    mask_d = nc.dram_tensor("masks", [P, P], BF, kind="ExternalInput")
    id_d = nc.dram_tensor("ident", [P, P], BF, kind="ExternalInput")
    out_d = nc.dram_tensor("outp", [S, DM], F32, kind="ExternalOutput")
    den_dram = nc.dram_tensor("den_scratch", [NHC, S], BF, kind="Internal")

    with tile.TileContext(nc) as tc:
        with (
            tc.tile_pool(name="const", bufs=1) as const,
            tc.tile_pool(name="resid", bufs=1) as resid,
        ):
            q_sb = resid.tile([P, NS, NHC * HD], BF)
            k_sb = resid.tile([P, NS, NHC * HD], BF)
            v_sb = resid.tile([P, NS, NHC, P], BF)  # per-head 128 cols (M pad)
            qtz = [resid.tile([P, S], BF, name=f"qtz{h}") for h in range(NHC)]
            ktz = [resid.tile([P, S], BF, name=f"ktz{h}") for h in range(NHC)]
            ao = [resid.tile([P, S], BF, name=f"ao{h}") for h in range(NHC)]
            den_rec = resid.tile([P, NHC, NS], F32)
            wo_sb = const.tile([P, NHC, DM], BF)  # rows HD:P zeroed (K pad)
            cos_sb = const.tile([P, NS, HD], BF)
            sin_sb = const.tile([P, NS, HD], BF)
            mask_sb = const.tile([P, P], BF)  # diag: 1 if p <= f else 0
            id_sb = const.tile([P, P], BF)

            # one-time zero/one fills (gpsimd, first in its queue)
            nc.gpsimd.memset(v_sb[:, :, :, HD], 1.0)       # denominator ones
            nc.gpsimd.memset(v_sb[:, :, :, HD + 1 : P], 0.0)  # M pad
            nc.gpsimd.memset(wo_sb[HD:P, :, :], 0.0)       # K pad (kills denom row)
            for h in range(NHC):
                nc.gpsimd.memset(ktz[h][HD:P, :], 0.0)     # K pad
                nc.gpsimd.memset(qtz[h][HD:P, :], 0.0)
                nc.gpsimd.memset(ao[h][HD:P, :], 0.0)  # K pad for outproj

            # all inputs pre-arranged on host: contiguous burst DMAs.
            # small tables first on scalar; xt + wqkv split across both HWDGEs.
            nc.scalar.dma_start(id_sb[:], id_d[:])
            nc.scalar.dma_start(mask_sb[:], mask_d[:])
            wqkv_sb = const.tile([P, NCC, QKV], BF)
            nc.sync.dma_start(wqkv_sb[:, 0:3, :], wqkv_d[:, 0:3, :])
            nc.scalar.dma_start(wqkv_sb[:, 3:NCC, :], wqkv_d[:, 3:NCC, :])
            xt_sb = const.tile([P, NCC, S], BF)
            for b in range(4):
                eng = nc.sync if b % 2 == 0 else nc.scalar
                eng.dma_start(xt_sb[:, :, b * XB : (b + 1) * XB], xt_d[b])
            nc.scalar.dma_start(cos_sb[:], cos_d[:])
            nc.scalar.dma_start(sin_sb[:], sin_d[:])
            nc.sync.dma_start(wo_sb[0:HD, :, :], wo_d[:])

            # ---- phase 1+2: qkv projection + rope + v pack + transposes ----
            with (
                tc.tile_pool(name="p1ps", bufs=3, space="PSUM") as pp,
                tc.tile_pool(name="p2ps", bufs=2, space="PSUM") as p2,
                tc.tile_pool(name="p1t", bufs=3) as tp,
            ):
                for s in range(NS):
                    pqkv = pp.tile([P, QKV], F32, tag="pqkv")
                    xsl = xt_sb[:, :, s * P : (s + 1) * P]
                    for c in range(NCC):
                        st, sp = (c == 0), (c == NCC - 1)
                        nc.tensor.matmul(
                            pqkv[:, 0:512], xsl[:, c, :], wqkv_sb[:, c, 0:512],
                            start=st, stop=sp,
                        )
                        nc.tensor.matmul(
                            pqkv[:, 512:QKV], xsl[:, c, :], wqkv_sb[:, c, 512:QKV],
                            start=st, stop=sp,
                        )
                    # rope on q and k together ([:, 0:384] of the psum tile).
                    # cos/sin tables hold one 64-wide head pattern; broadcast
                    # over the 6 head-copies with a step-0 AP dim.
                    qk = pqkv[:, 0:384]
                    qk3 = qk.rearrange("p (b x) -> p b x", x=32)
                    cs = cos_sb[:, s, :]
                    sn = sin_sb[:, s, :]
                    c3b = bass.AP(cs.tensor, cs.offset, [cs.ap[0], [0, 6], [1, HD]])
                    s3a = bass.AP(sn.tensor, sn.offset, [sn.ap[0], [0, 6], [32, 2], [1, 16]])
                    s3b = bass.AP(sn.tensor, sn.offset + 16, [sn.ap[0], [0, 6], [32, 2], [1, 16]])
                    t = tp.tile([P, 384], F32, tag="ropet")
                    t3 = t.rearrange("p (b x) -> p b x", x=32)
                    # t = shuffle(qk) * sin_signed  (swap 16-halves per 32-block)
                    nc.vector.tensor_tensor(
                        t3[:, :, 0:16], qk3[:, :, 16:32], s3a, ALU.mult
                    )
                    nc.vector.tensor_tensor(
                        t3[:, :, 16:32], qk3[:, :, 0:16], s3b, ALU.mult
                    )
                    t2 = tp.tile([P, 384], F32, tag="ropet2")
                    nc.vector.tensor_tensor(t2[:], qk[:], c3b, ALU.mult)
                    # final add on gpsimd (sbuf-only engine), cast to bf16
                    nc.gpsimd.tensor_tensor(
                        q_sb[:, s, :], t2[:, 0:192], t[:, 0:192], ALU.add
                    )
                    nc.gpsimd.tensor_tensor(
                        k_sb[:, s, :], t2[:, 192:384], t[:, 192:384], ALU.add
                    )
                    # v pack with ones columns
                    nc.scalar.copy(
                        v_sb[:, s, :, 0:HD],
                        pqkv[:, 384:QKV].rearrange("p (h x) -> p h x", x=HD),
                    )
                    # transposes to d-major, per-head zero-padded layout
                    sl = slice(s * P, (s + 1) * P)
                    for src_t, dst, ev in ((k_sb, ktz, "v"), (q_sb, qtz, "a")):
                        pt = p2.tile([P, 2 * P], BF, tag="pt")
                        nc.tensor.transpose(pt[:, 0:P], src_t[:, s, 0:P], id_sb[:])
                        nc.tensor.transpose(
                            pt[0:HD, P : 2 * P], src_t[:, s, P : P + HD], id_sb[:]
                        )
                        eng = nc.scalar.copy if ev == "a" else nc.vector.tensor_copy
                        eng(dst[0][0:HD, sl], pt[0:HD, 0:P])
                        eng(dst[1][0:HD, sl], pt[HD:P, 0:P])
                        eng(dst[2][0:HD, sl], pt[0:HD, P : 2 * P])

            # ---- phase 3+4: attention with software-pipelined outproj ----
            # attention runs qj-outer / h-inner; the output projection of
            # block qj is emitted during block qj+1's attention so its
            # denominator DMA round-trip and PSUM evictions hide under PE work.
            with (
                tc.tile_pool(name="scps", bufs=3, space="PSUM") as scp,
                tc.tile_pool(name="aops", bufs=1, space="PSUM") as aop,
                tc.tile_pool(name="p4ps", bufs=2, space="PSUM") as p4,
                tc.tile_pool(name="expp", bufs=10) as expp,
                tc.tile_pool(name="outp", bufs=3) as op,
            ):
                den_sb = resid.tile([P, NHC, NS], BF)

                def attention_block(qj):
                    qsl = slice(qj * SQT, (qj + 1) * SQT)
                    nki = KPQ * qj + KPQ
                    for h in range(NHC):
                        kth, qth = ktz[h], qtz[h]
                        pa = aop.tile([P, SQT], F32, tag="pa")
                        for ki in range(nki):
                            r = ki - KPQ * qj  # >= 0: diagonal-crossing tile
                            off = max(r, 0) * P
                            ps = scp.tile([P, SQT], F32, tag="ps")
                            nc.tensor.matmul(
                                ps[:, off:SQT],
                                kth[:, ki * P : (ki + 1) * P],
                                qth[:, qj * SQT + off : (qj + 1) * SQT],
                                start=True, stop=True,
                            )
                            e = expp.tile([P, SQT], BF, tag="e")
                            nc.scalar.activation(
                                e[:, off:SQT], ps[:, off:SQT], AF.Exp, scale=0.125
                            )
                            if r >= 0:
                                if off > 0:
                                    nc.gpsimd.memset(e[:, 0:off], 0.0)
                                nc.gpsimd.tensor_tensor(
                                    e[:, off : off + P],
                                    e[:, off : off + P],
                                    mask_sb[:],
                                    ALU.mult,
                                )
                            nc.tensor.matmul(
                                pa[:],
                                v_sb[:, ki, h, :],
                                e[:],
                                start=(ki == 0), stop=(ki == nki - 1),
                            )
                        nc.vector.tensor_copy(ao[h][0 : HD + 1, qsl], pa[0 : HD + 1, :])
                        # denominator row -> DRAM (re-read s-major below)
                        nc.sync.dma_start(den_dram[h, qsl], ao[h][HD : HD + 1, qsl])
                    csl = slice(qj * KPQ, (qj + 1) * KPQ)
                    for h in range(NHC):
                        nc.sync.dma_start(
                            den_sb[:, h, csl],
                            den_dram[h, qsl].rearrange("(n p) -> p n", p=P),
                        )
                    nc.vector.reciprocal(den_rec[:, :, csl], den_sb[:, :, csl])

                def outproj_block(qj):
                    for s in range(qj * KPQ, (qj + 1) * KPQ):
                        sl = slice(s * P, (s + 1) * P)
                        acc = op.tile([P, DM], F32, tag="acc")
                        for h in range(NHC):
                            po = p4.tile([P, DM], F32, tag="po")
                            lh = ao[h][:, sl]  # K=128: denom row killed by wo zeros
                            nc.tensor.matmul(
                                po[:, 0:512], lh, wo_sb[:, h, 0:512],
                                start=True, stop=True,
                            )
                            nc.tensor.matmul(
                                po[:, 512:DM], lh, wo_sb[:, h, 512:DM],
                                start=True, stop=True,
                            )
                            scale = den_rec[:, h, s : s + 1]
                            if h == 0:
                                nc.vector.tensor_scalar_mul(acc[:], po[:], scale)
                            else:
                                nc.vector.scalar_tensor_tensor(
                                    acc[:], po[:], scale, acc[:], ALU.mult, ALU.add
                                )
                        nc.sync.dma_start(out_d[sl, :], acc[:])

                for qj in range(NQ):
                    attention_block(qj)
                    if qj >= 1:
                        outproj_block(qj - 1)
                outproj_block(NQ - 1)

    nc.compile()
    return nc


_cache = {}
LAST_RESULT = None


def _get_program(S, n_devices):
    key = (S, n_devices)
    if key not in _cache:
        _cache[key] = build_program(S, n_devices)
    return _cache[key]


def _rope_tables(row_ids, col_ids, S):
    inv = 1.0 / (10000.0 ** (np.arange(0, 32, 2, dtype=np.float64) / 32.0))

    def block(ids):
        ang = ids.astype(np.float64)[:, None] * inv[None, :]
        c = np.concatenate([np.cos(ang), np.cos(ang)], -1)
        s_ = np.concatenate([-np.sin(ang), np.sin(ang)], -1)  # signed (shuffle form)
        return c, s_

    cr, sr = block(np.asarray(row_ids))
    cc, sc = block(np.asarray(col_ids))
    cos64 = np.concatenate([cr, cc], -1)
    sin64 = np.concatenate([sr, sc], -1)
    return cos64.astype(bfloat16), sin64.astype(bfloat16)


def _make_masks():
    pp_ = np.arange(P)[:, None]
    ff = np.arange(P)[None, :]
    return (pp_ <= ff).astype(np.float32).astype(bfloat16)


def kernel(x, row_ids, col_ids, Wq, Wk, Wv, Wo):
    x = np.asarray(x)
    B, S, _ = x.shape
    n_cores = 8
    groups = n_cores // B  # head groups per batch (4)
    hpg = NHC  # heads per group

    nc = _get_program(S, n_cores)
    cos_t, sin_t = _rope_tables(row_ids, col_ids, S)
    cos_t = np.ascontiguousarray(cos_t.reshape(S // P, P, -1).transpose(1, 0, 2))
    sin_t = np.ascontiguousarray(sin_t.reshape(S // P, P, -1).transpose(1, 0, 2))
    masks = _make_masks()
    ident = np.eye(P, dtype=bfloat16)

    Wq, Wk, Wv, Wo = (np.asarray(w, np.float32) for w in (Wq, Wk, Wv, Wo))
    in_maps = []
    for c in range(n_cores):
        b = c // groups
        h0 = (c % groups) * hpg
        rows = slice(h0 * HD, (h0 + hpg) * HD)
        xt = np.ascontiguousarray(x[b].T).astype(bfloat16)
        NXB = S // 512
        xt = np.ascontiguousarray(
            xt.reshape(NCC, P, NXB, 512).transpose(2, 1, 0, 3)
        )
        wqkv = np.concatenate(
            [Wq[rows].T, Wk[rows].T, Wv[rows].T], axis=1
        ).astype(bfloat16)
        wqkv = np.ascontiguousarray(wqkv.reshape(NCC, P, QKV).transpose(1, 0, 2))
        wo = np.ascontiguousarray(Wo[:, rows].T).astype(bfloat16)
        wo = np.ascontiguousarray(wo.reshape(NHC, HD, DM).transpose(1, 0, 2))
        in_maps.append(
            {
                "xt": xt,
                "wqkv": wqkv,
                "wo": wo,
                "cos": cos_t,
                "sin": sin_t,
                "masks": masks,
                "ident": ident,
            }
        )

    import os

    trace = bool(os.environ.get("KERNEL_TRACE"))
    kw = {}
    if trace and os.environ.get("KERNEL_TRACE_DIR"):
        kw["tmpdir"] = os.environ["KERNEL_TRACE_DIR"]
    res = run_bass_kernel_spmd(nc, in_maps, list(range(n_cores)), trace=trace, **kw)
    global LAST_RESULT
    LAST_RESULT = res

    outs = [res.results[c]["outp"] for c in range(n_cores)]
    out = np.stack(
        [sum(outs[b * groups + g] for g in range(groups)) for b in range(B)], axis=0
    )
    return out.astype(np.float32)


# revision 43
# speedup vs baseline: 1.0818x; 1.0818x over previous
"""Trainium2 Bass kernel for causal self-attention with 2D RoPE.

Sharding: batch x head-group parallel over 8 NeuronCores.
  core c -> batch b = c // 4, heads h0 = (c % 4) * 3 .. h0+2.
Each core computes q/k/v projections for its 3 heads, 2D RoPE, causal
flash-attention (transposed-score layout, denominator via an appended
ones-column on V), and a per-head output projection with the softmax
normalization folded into the PSUM eviction scale. The host sums the
4 partial outputs per batch.

Matmuls run in bf16 (fp32 PSUM accumulation). All matmuls are zero-padded
to K=128 contraction so the PE activity monitor keeps the clock at 2.4GHz.
"""

import sys

sys.path.insert(0, "/opt/trn_rl_repo")

import numpy as np
from ml_dtypes import bfloat16

import concourse.bacc as bacc
import concourse.bass as bass
import concourse.mybir as mybir
from concourse import tile
from concourse.bass_utils import run_bass_kernel_spmd

BF = mybir.dt.bfloat16
F32 = mybir.dt.float32
AF = mybir.ActivationFunctionType
ALU = mybir.AluOpType

P = 128          # partitions
DM = 768         # d_model
HD = 64          # head dim
NHC = 3          # heads per core
NCC = DM // P    # contraction chunks (6)
SQT = 512        # q-block (matmul moving dim)
QKV = 3 * NHC * HD  # 576


def build_program(S=2048, n_devices=8):
    NS = S // P      # seq chunks of 128
    NQ = S // SQT    # q blocks of 512
    KPQ = SQT // P   # k-chunks per q-block (4)

    nc = bacc.Bacc(
        "TRN2", target_bir_lowering=False, debug=False, num_devices=n_devices
    )
    XB = 512
    NXB = S // XB
    xt_d = nc.dram_tensor("xt", [NXB, P, NCC, XB], BF, kind="ExternalInput")
    wqkv_d = nc.dram_tensor("wqkv", [P, NCC, QKV], BF, kind="ExternalInput")
    wo_d = nc.dram_tensor("wo", [HD, NHC, DM], BF, kind="ExternalInput")
    cos_d = nc.dram_tensor("cos", [P, S // P, HD], BF, kind="ExternalInput")
    sin_d = nc.dram_tensor("sin", [P, S // P, HD], BF, kind="ExternalInput")
    mask_d = nc.dram_tensor("masks", [P, P], BF, kind="ExternalInput")
    id_d = nc.dram_tensor("ident", [P, P], BF, kind="ExternalInput")
    out_d = nc.dram_tensor("outp", [S, DM], F32, kind="ExternalOutput")
    den_dram = nc.dram_tensor("den_scratch", [NHC, S], BF, kind="Internal")

    with tile.TileContext(nc) as tc:
        with (
            tc.tile_pool(name="const", bufs=1) as const,
            tc.tile_pool(name="resid", bufs=1) as resid,
        ):
            q_sb = resid.tile([P, NS, NHC * HD], BF)
            k_sb = resid.tile([P, NS, NHC * HD], BF)
            v_sb = resid.tile([P, NS, NHC, P], BF)  # per-head 128 cols (M pad)
            qtz = [resid.tile([P, S], BF, name=f"qtz{h}") for h in range(NHC)]
            ktz = [resid.tile([P, S], BF, name=f"ktz{h}") for h in range(NHC)]
            ao = [resid.tile([P, S], BF, name=f"ao{h}") for h in range(NHC)]
            den_rec = resid.tile([P, NHC, NS], F32)
            wo_sb = const.tile([P, NHC, DM], BF)  # rows HD:P zeroed (K pad)
            cos_sb = const.tile([P, NS, HD], BF)
            sin_sb = const.tile([P, NS, HD], BF)
            mask_sb = const.tile([P, P], BF)  # diag: 1 if p <= f else 0
            id_sb = const.tile([P, P], BF)

            # one-time zero/one fills (gpsimd, first in its queue)
            nc.gpsimd.memset(v_sb[:, :, :, HD], 1.0)       # denominator ones
            nc.gpsimd.memset(v_sb[:, :, :, HD + 1 : P], 0.0)  # M pad
            nc.gpsimd.memset(wo_sb[HD:P, :, :], 0.0)       # K pad (kills denom row)
            for h in range(NHC):
                nc.gpsimd.memset(ktz[h][HD:P, :], 0.0)     # K pad
                nc.gpsimd.memset(qtz[h][HD:P, :], 0.0)
                nc.gpsimd.memset(ao[h][HD:P, :], 0.0)  # K pad for outproj

            # all inputs pre-arranged on host: contiguous burst DMAs.
            # small tables first on scalar; xt + wqkv split across both HWDGEs.
            nc.scalar.dma_start(id_sb[:], id_d[:])
            nc.scalar.dma_start(mask_sb[:], mask_d[:])
            wqkv_sb = const.tile([P, NCC, QKV], BF)
            nc.sync.dma_start(wqkv_sb[:, 0:3, :], wqkv_d[:, 0:3, :])
            nc.scalar.dma_start(wqkv_sb[:, 3:NCC, :], wqkv_d[:, 3:NCC, :])
            xt_sb = const.tile([P, NCC, S], BF)
            for b in range(4):
                eng = nc.sync if b % 2 == 0 else nc.scalar
                eng.dma_start(xt_sb[:, :, b * XB : (b + 1) * XB], xt_d[b])
            nc.scalar.dma_start(cos_sb[:], cos_d[:])
            nc.scalar.dma_start(sin_sb[:], sin_d[:])
            nc.sync.dma_start(wo_sb[0:HD, :, :], wo_d[:])

            # ---- phase 1+2: qkv projection + rope + v pack + transposes ----
            with (
                tc.tile_pool(name="p1ps", bufs=3, space="PSUM") as pp,
                tc.tile_pool(name="p2ps", bufs=2, space="PSUM") as p2,
                tc.tile_pool(name="p1t", bufs=3) as tp,
            ):
                for s in range(NS):
                    pqkv = pp.tile([P, QKV], F32, tag="pqkv")
                    xsl = xt_sb[:, :, s * P : (s + 1) * P]
                    for c in range(NCC):
                        st, sp = (c == 0), (c == NCC - 1)
                        nc.tensor.matmul(
                            pqkv[:, 0:512], xsl[:, c, :], wqkv_sb[:, c, 0:512],
                            start=st, stop=sp,
                        )
                        nc.tensor.matmul(
                            pqkv[:, 512:QKV], xsl[:, c, :], wqkv_sb[:, c, 512:QKV],
                            start=st, stop=sp,
                        )
                    # rope on q and k together ([:, 0:384] of the psum tile).
                    # cos/sin tables hold one 64-wide head pattern; broadcast
                    # over the 6 head-copies with a step-0 AP dim.
                    qk = pqkv[:, 0:384]
                    qk3 = qk.rearrange("p (b x) -> p b x", x=32)
                    cs = cos_sb[:, s, :]
                    sn = sin_sb[:, s, :]
                    c3b = bass.AP(cs.tensor, cs.offset, [cs.ap[0], [0, 6], [1, HD]])
                    s3a = bass.AP(sn.tensor, sn.offset, [sn.ap[0], [0, 6], [32, 2], [1, 16]])
                    s3b = bass.AP(sn.tensor, sn.offset + 16, [sn.ap[0], [0, 6], [32, 2], [1, 16]])
                    t = tp.tile([P, 384], F32, tag="ropet")
                    t3 = t.rearrange("p (b x) -> p b x", x=32)
                    # t = shuffle(qk) * sin_signed  (swap 16-halves per 32-block)
                    nc.vector.tensor_tensor(
                        t3[:, :, 0:16], qk3[:, :, 16:32], s3a, ALU.mult
                    )
                    nc.vector.tensor_tensor(
                        t3[:, :, 16:32], qk3[:, :, 0:16], s3b, ALU.mult
                    )
                    t2 = tp.tile([P, 384], F32, tag="ropet2")
                    nc.vector.tensor_tensor(t2[:], qk[:], c3b, ALU.mult)
                    # final add on gpsimd (sbuf-only engine), cast to bf16
                    nc.gpsimd.tensor_tensor(
                        q_sb[:, s, :], t2[:, 0:192], t[:, 0:192], ALU.add
                    )
                    nc.gpsimd.tensor_tensor(
                        k_sb[:, s, :], t2[:, 192:384], t[:, 192:384], ALU.add
                    )
                    # v pack with ones columns
                    nc.scalar.copy(
                        v_sb[:, s, :, 0:HD],
                        pqkv[:, 384:QKV].rearrange("p (h x) -> p h x", x=HD),
                    )
                    # transposes to d-major, per-head zero-padded layout
                    sl = slice(s * P, (s + 1) * P)
                    for src_t, dst, ev in ((k_sb, ktz, "v"), (q_sb, qtz, "a")):
                        pt = p2.tile([P, 2 * P], BF, tag="pt")
                        nc.tensor.transpose(pt[:, 0:P], src_t[:, s, 0:P], id_sb[:])
                        nc.tensor.transpose(
                            pt[0:HD, P : 2 * P], src_t[:, s, P : P + HD], id_sb[:]
                        )
                        eng = nc.scalar.copy if ev == "a" else nc.vector.tensor_copy
                        eng(dst[0][0:HD, sl], pt[0:HD, 0:P])
                        eng(dst[1][0:HD, sl], pt[HD:P, 0:P])
                        eng(dst[2][0:HD, sl], pt[0:HD, P : 2 * P])

            # ---- phase 3+4: attention with software-pipelined outproj ----
            # attention runs qj-outer / h-inner; the output projection of
            # block qj is emitted during block qj+1's attention so its
            # denominator DMA round-trip and PSUM evictions hide under PE work.
            with (
                tc.tile_pool(name="scps", bufs=3, space="PSUM") as scp,
                tc.tile_pool(name="aops", bufs=1, space="PSUM") as aop,
                tc.tile_pool(name="p4ps", bufs=2, space="PSUM") as p4,
                tc.tile_pool(name="expp", bufs=10) as expp,
                tc.tile_pool(name="outp", bufs=3) as op,
            ):
                den_sb = resid.tile([P, NHC, NS], BF)

                def attention_block(qj):
                    qsl = slice(qj * SQT, (qj + 1) * SQT)
                    nki = KPQ * qj + KPQ
                    for h in range(NHC):
                        kth, qth = ktz[h], qtz[h]
                        pa = aop.tile([P, SQT], F32, tag="pa")
                        for ki in range(nki):
                            r = ki - KPQ * qj  # >= 0: diagonal-crossing tile
                            off = max(r, 0) * P
                            ps = scp.tile([P, SQT], F32, tag="ps")
                            nc.tensor.matmul(
                                ps[:, off:SQT],
                                kth[:, ki * P : (ki + 1) * P],
                                qth[:, qj * SQT + off : (qj + 1) * SQT],
                                start=True, stop=True,
                            )
                            e = expp.tile([P, SQT], BF, tag="e")
                            nc.scalar.activation(
                                e[:, off:SQT], ps[:, off:SQT], AF.Exp, scale=0.125
                            )
                            if r >= 0:
                                if off > 0:
                                    nc.vector.memset(e[:, 0:off], 0.0)
                                nc.vector.tensor_tensor(
                                    e[:, off : off + P],
                                    e[:, off : off + P],
                                    mask_sb[:],
                                    ALU.mult,
                                )
                            nc.tensor.matmul(
                                pa[:],
                                v_sb[:, ki, h, :],
                                e[:],
                                start=(ki == 0), stop=(ki == nki - 1),
                            )
                        nc.vector.tensor_copy(ao[h][0 : HD + 1, qsl], pa[0 : HD + 1, :])
                        # denominator row -> DRAM (re-read s-major below)
                        nc.sync.dma_start(den_dram[h, qsl], ao[h][HD : HD + 1, qsl])
                    csl = slice(qj * KPQ, (qj + 1) * KPQ)
                    for h in range(NHC):
                        nc.sync.dma_start(
                            den_sb[:, h, csl],
                            den_dram[h, qsl].rearrange("(n p) -> p n", p=P),
                        )
                    nc.vector.reciprocal(den_rec[:, :, csl], den_sb[:, :, csl])

                def outproj_block(qj):
                    for s in range(qj * KPQ, (qj + 1) * KPQ):
                        sl = slice(s * P, (s + 1) * P)
                        acc = op.tile([P, DM], F32, tag="acc")
                        for h in range(NHC):
                            po = p4.tile([P, DM], F32, tag="po")
                            lh = ao[h][:, sl]  # K=128: denom row killed by wo zeros
                            nc.tensor.matmul(
                                po[:, 0:512], lh, wo_sb[:, h, 0:512],
                                start=True, stop=True,
                            )
                            nc.tensor.matmul(
                                po[:, 512:DM], lh, wo_sb[:, h, 512:DM],
                                start=True, stop=True,
                            )
                            scale = den_rec[:, h, s : s + 1]
                            if h == 0:
                                nc.vector.tensor_scalar_mul(acc[:], po[:], scale)
                            else:
                                nc.vector.scalar_tensor_tensor(
                                    acc[:], po[:], scale, acc[:], ALU.mult, ALU.add
                                )
                        nc.sync.dma_start(out_d[sl, :], acc[:])

                for qj in range(NQ):
                    attention_block(qj)
                    if qj >= 1:
                        outproj_block(qj - 1)
                outproj_block(NQ - 1)

    nc.compile()
    return nc


_cache = {}
LAST_RESULT = None


def _get_program(S, n_devices):
    key = (S, n_devices)
    if key not in _cache:
        _cache[key] = build_program(S, n_devices)
    return _cache[key]


def _rope_tables(row_ids, col_ids, S):
    inv = 1.0 / (10000.0 ** (np.arange(0, 32, 2, dtype=np.float64) / 32.0))

    def block(ids):
        ang = ids.astype(np.float64)[:, None] * inv[None, :]
        c = np.concatenate([np.cos(ang), np.cos(ang)], -1)
        s_ = np.concatenate([-np.sin(ang), np.sin(ang)], -1)  # signed (shuffle form)
        return c, s_

    cr, sr = block(np.asarray(row_ids))
    cc, sc = block(np.asarray(col_ids))
    cos64 = np.concatenate([cr, cc], -1)
    sin64 = np.concatenate([sr, sc], -1)
    return cos64.astype(bfloat16), sin64.astype(bfloat16)


def _make_masks():
    pp_ = np.arange(P)[:, None]
    ff = np.arange(P)[None, :]
    return (pp_ <= ff).astype(np.float32).astype(bfloat16)


def kernel(x, row_ids, col_ids, Wq, Wk, Wv, Wo):
    x = np.asarray(x)
    B, S, _ = x.shape
    n_cores = 8
    groups = n_cores // B  # head groups per batch (4)
    hpg = NHC  # heads per group

    nc = _get_program(S, n_cores)
    cos_t, sin_t = _rope_tables(row_ids, col_ids, S)
    cos_t = np.ascontiguousarray(cos_t.reshape(S // P, P, -1).transpose(1, 0, 2))
    sin_t = np.ascontiguousarray(sin_t.reshape(S // P, P, -1).transpose(1, 0, 2))
    masks = _make_masks()
    ident = np.eye(P, dtype=bfloat16)

    Wq, Wk, Wv, Wo = (np.asarray(w, np.float32) for w in (Wq, Wk, Wv, Wo))
    in_maps = []
    for c in range(n_cores):
        b = c // groups
        h0 = (c % groups) * hpg
        rows = slice(h0 * HD, (h0 + hpg) * HD)
        xt = np.ascontiguousarray(x[b].T).astype(bfloat16)
        NXB = S // 512
        xt = np.ascontiguousarray(
            xt.reshape(NCC, P, NXB, 512).transpose(2, 1, 0, 3)
        )
        wqkv = np.concatenate(
            [Wq[rows].T, Wk[rows].T, Wv[rows].T], axis=1
        ).astype(bfloat16)
        wqkv = np.ascontiguousarray(wqkv.reshape(NCC, P, QKV).transpose(1, 0, 2))
        wo = np.ascontiguousarray(Wo[:, rows].T).astype(bfloat16)
        wo = np.ascontiguousarray(wo.reshape(NHC, HD, DM).transpose(1, 0, 2))
        in_maps.append(
            {
                "xt": xt,
                "wqkv": wqkv,
                "wo": wo,
                "cos": cos_t,
                "sin": sin_t,
                "masks": masks,
                "ident": ident,
            }
        )

    import os

    trace = bool(os.environ.get("KERNEL_TRACE"))
    kw = {}
    if trace and os.environ.get("KERNEL_TRACE_DIR"):
        kw["tmpdir"] = os.environ["KERNEL_TRACE_DIR"]
    res = run_bass_kernel_spmd(nc, in_maps, list(range(n_cores)), trace=trace, **kw)
    global LAST_RESULT
    LAST_RESULT = res

    outs = [res.results[c]["outp"] for c in range(n_cores)]
    out = np.stack(
        [sum(outs[b * groups + g] for g in range(groups)) for b in range(B)], axis=0
    )
    return out.astype(np.float32)
